# revision 4
# baseline (speedup 1.0000x reference)
"""LoRA attention kernel for Trainium2, batch-sharded across 8 NeuronCores.

Strategy (v4):
  - Data parallel: batch B=8 -> one batch element per core.
  - LoRA factors folded into Wqkv on the host (exact algebra, float64);
    the 1/sqrt(hd) score scale is folded into Wq as well.
  - All matmul operands are bfloat16 (1 cycle/row on the PE regardless of
    moving free dim, and FWL halves LDWEIGHTS time); accumulation is fp32
    in PSUM so only operand rounding is lost.
  - Heads are processed in pairs (2p, 2p+1). qT/kT tiles pack the pair's
    head dims on partitions [0:64] / [64:128]. The score matmuls are ROW
    TILED: two concurrent 64-row matmuls (tile_position auto-derived from
    base partitions) compute both heads' scores in one 512-column pass.
  - The pair's two score outputs go to one [128,1024] 2-bank PSUM tile so
    a single ACT instruction applies exp to both (halves ACT bubbles).
    The exp stream is the pacing engine (~107us); everything else is
    arranged to hide under it:
      * weights live in DRAM as [128, 768] all-pair tiles so lead-in DMAs
        move 1.5KB per-partition lines; pair-0 column slices go first,
        split across the sync and gpsimd DMA queues.
      * v-projection groups are interleaved with pair-0/qc-0 attention —
        the attention-value matmul for key tile kt only needs vaug[kt].
      * qk-projection for pair p+1 is emitted between the two query
        halves of pair p.
      * the attention kt loop is software-pipelined: scores/exp run two
        key tiles ahead of the attention-value matmuls, so the av PSUM
        bank turnaround at (pair, qc) boundaries does not stall the
        in-order PE queue.
      * pair 5 handles its second query half first, and the output
        projection for those tokens is emitted right after, so it fills
        PE gaps while pair-5/qc-0 attention is still ACT-paced.
  - v is produced in natural layout with an extra all-ones column per head
    (65-col pitch); the attention-value matmul (M=65) accumulates softmax
    denominators for free in its last output row.
  - Normalization runs off the PE critical path: DVE drains av PSUM, the
    gpsimd DMA queue shifts the denominator row to partition 0, fast
    reciprocal, gpsimd partition-broadcast, DVE multiply into outT.
  - y is written back as bf16 (cast to f32 on host).
"""
import numpy as np
import ml_dtypes

import concourse.bass as bass
import concourse.bacc as bacc
import concourse.mybir as mybir
import concourse.tile as tile
from concourse.bass_utils import run_bass_kernel_spmd

F32 = mybir.dt.float32
BF16 = mybir.dt.bfloat16
EXP = mybir.ActivationFunctionType.Exp

B, N, C, H, HD = 8, 1024, 768, 12, 64
P = H // 2              # 6 head pairs
CT = C // 128           # 6 contraction tiles over C
QC = N // 512           # 2 query chunks of 512
KT = N // 128           # 8 key tiles of 128
EC = 2                  # output-projection feature chunks of 384
VP = HD + 1             # 65-col pitch per head in vaug
SKEW = 2                # score/exp lead over av in the kt pipeline
N_CORES = 8

_NC_CACHE = None


def _build():
    nc = bacc.Bacc(None, target_bir_lowering=False)

    xT = nc.dram_tensor("xT", [C, N], BF16, kind="ExternalInput")
    # wq/wk[c] = [K=c-rows(128), 6 pairs x (head 2p dims 64 | head 2p+1 dims 64)]
    wq = nc.dram_tensor("wq", [CT, 128, C], BF16, kind="ExternalInput")
    wk = nc.dram_tensor("wk", [CT, 128, C], BF16, kind="ExternalInput")
    wv = nc.dram_tensor("wv", [CT, 128, C], BF16, kind="ExternalInput")
    wpt = nc.dram_tensor("wpt", [CT, 128, C], BF16, kind="ExternalInput")
    bias = nc.dram_tensor("bias", [1, C], F32, kind="ExternalInput")
    y = nc.dram_tensor("y", [N, C], BF16, kind="ExternalOutput")

    from contextlib import ExitStack
    with tile.TileContext(nc) as tc:
        with ExitStack() as ctx:
            pool = lambda name, bufs, **kw: ctx.enter_context(
                tc.tile_pool(name=name, bufs=bufs, **kw))
            xt_pool = pool("xt", 2 * CT)
            wq_pool = pool("wqp", CT)             # resident all-pair tiles
            wk_pool = pool("wkp", CT)
            w768_pool = pool("w768", 2 * CT)      # wv then wpt (disjoint phases)
            vaug_pool = pool("vaug", KT)
            qt_pool = pool("qtp", 4)              # qT pair tiles, 2 pairs
            kt_pool = pool("ktp", 4)
            et_pool = pool("expp", 6)
            avs_pool = pool("avsp", 4)
            iv_pool = pool("ivp", 4)
            bc_pool = pool("bcp", 3)
            ost_pool = pool("ostp", 3)
            out_pool = pool("outp", 2 * CT)
            y_pool = pool("yp", 3)
            cst_pool = pool("cst", 1)
            proj_ps = pool("proj_ps", 2, space="PSUM")
            sc_ps = pool("sc_ps", 2, space="PSUM")
            av_ps = pool("av_ps", 2, space="PSUM")

            # ---- PE warm-up: dummy matmuls bridge the DMA lead-in so the
            # HAM clock gate opens before real work arrives -----------------
            wuf = cst_pool.tile([128, 512], F32, tag="wuf")
            nc.vector.memset(wuf, 0.0)
            wur = cst_pool.tile([128, 512], BF16, tag="wur")
            nc.vector.tensor_copy(wur, wuf)

            def warmup(n, label):
                for i in range(n):
                    wps = proj_ps.tile([128, 512], F32, tag="mmps",
                                       name=f"wu{label}_{i}")
                    nc.tensor.matmul(wps, wur[:, 0:128], wur,
                                     start=True, stop=True)

            warmup(14, "a")

            # ---- loads: pair-0 weight slices first, split across queues --
            wqt = [wq_pool.tile([128, C], BF16, tag="wq", name=f"wq{c}")
                   for c in range(CT)]
            wkt = [wk_pool.tile([128, C], BF16, tag="wk", name=f"wk{c}")
                   for c in range(CT)]
            for c in range(CT):
                nc.sync.dma_start(out=wqt[c][:, 0:128], in_=wq[c, :, 0:128])
                nc.gpsimd.dma_start(out=wkt[c][:, 0:128], in_=wk[c, :, 0:128])

            # x^T in 12 standalone [128, 512] bf16 tiles; qc0 on the sync
            # queue (critical for first scores), qc1 on gpsimd's queue
            xt = [[None] * QC for _ in range(CT)]
            for qc in range(QC):
                for c in range(CT):
                    t = xt_pool.tile([128, 512], BF16, tag="xt",
                                     name=f"xt{c}_{qc}")
                    eng = nc.sync if qc == 0 else nc.gpsimd
                    eng.dma_start(
                        out=t, in_=xT[c * 128:(c + 1) * 128,
                                      qc * 512:(qc + 1) * 512])
                    xt[c][qc] = t

            # remaining weight columns (pairs 1-5), 1.25KB lines
            for c in range(CT):
                nc.sync.dma_start(out=wqt[c][:, 128:C], in_=wq[c, :, 128:C])
                nc.gpsimd.dma_start(out=wkt[c][:, 128:C], in_=wk[c, :, 128:C])

            bias_bc = cst_pool.tile([128, C], F32, tag="biasbc")
            nc.gpsimd.dma_start(out=bias_bc,
                                in_=bias[:, :].to_broadcast([128, C]))
            ones12 = cst_pool.tile([128, H], BF16, tag="ones12")
            nc.vector.memset(ones12, 1.0)

            wvt = []
            for c in range(CT):
                t = w768_pool.tile([128, C], BF16, tag="w768",
                                   name=f"wv{c}")
                nc.gpsimd.dma_start(out=t, in_=wv[c, :, :])
                wvt.append(t)

            # ---- per-pair q/k projection (qc0 groups first so the first
            # scores can start two groups earlier) -------------------------
            def qk_project(p):
                """qT_pair / kT_pair: [128 = (h dims | h' dims), 512 tokens]
                per 512-token chunk, bf16."""
                qts, kts = [None] * QC, [None] * QC
                for qc in range(QC):
                    for qk in range(2):
                        dst_pool = qt_pool if qk == 0 else kt_pool
                        wts = wqt if qk == 0 else wkt
                        st = dst_pool.tile([128, 512], BF16, tag="st",
                                           name=f"st{p}_{qk}_{qc}")
                        pqk = proj_ps.tile([128, 512], F32, tag="mmps",
                                           name=f"pqk{p}_{qk}_{qc}")
                        for c in range(CT):
                            nc.tensor.matmul(
                                pqk, wts[c][:, p * 128:(p + 1) * 128],
                                xt[c][qc],
                                start=(c == 0), stop=(c == CT - 1),
                            )
                        nc.vector.tensor_copy(st, pqk)
                        (qts if qk == 0 else kts)[qc] = st
                return qts, kts

            # ---- v_aug[kt] group emission (natural layout, [v | 1]) ------
            vaug = [None] * KT

            def vproj(tt):
                va = vaug_pool.tile([128, H * VP], BF16,
                                    tag="vaug", name=f"vaug{tt}")
                for half in range(2):
                    pv = proj_ps.tile([128, 384], F32, tag="mmps",
                                      name=f"pv{tt}_{half}")
                    for c in range(CT):
                        nc.tensor.matmul(
                            pv,
                            xt[c][tt // 4][:, (tt % 4) * 128:(tt % 4 + 1) * 128],
                            wvt[c][:, half * 384:(half + 1) * 384],
                            start=(c == 0), stop=(c == CT - 1),
                        )
                    dst = bass.AP(tensor=va.tensor,
                                  offset=va.offset + half * 6 * VP,
                                  ap=[va.ap[0], [VP, 6], [1, HD]])
                    nc.vector.tensor_copy(dst, pv)
                ones_ap = bass.AP(tensor=va.tensor, offset=va.offset + HD,
                                  ap=[va.ap[0], [VP, H]])
                nc.vector.tensor_copy(ones_ap, ones12)
                vaug[tt] = va

            # ---- output accumulator tiles (c-major, [128, 512] per qc) ---
            outT = [[out_pool.tile([128, 512], BF16, tag="outT",
                                   name=f"outT{i}_{qc}")
                     for qc in range(QC)] for i in range(CT)]

            # ---- attention building blocks -------------------------------
            def score_exp(p, qc, qts, kts, kt):
                ps_s = sc_ps.tile([128, 1024], F32, tag="sc",
                                  name=f"sc{p}_{qc}_{kt}")
                klhs = kts[kt // 4][:, (kt % 4) * 128:(kt % 4 + 1) * 128]
                nc.tensor.matmul(
                    ps_s[:, 0:512], klhs[0:64, :], qts[qc][0:64, :],
                    start=True, stop=True,
                )
                nc.tensor.matmul(
                    ps_s[:, 512:1024], klhs[64:128, :], qts[qc][64:128, :],
                    start=True, stop=True,
                )
                et = et_pool.tile([128, 1024], BF16, tag="exp",
                                  name=f"exp{p}_{qc}_{kt}")
                nc.scalar.activation(out=et, in_=ps_s, func=EXP)
                return et

            def av_step(p, av0, av1, et, kt):
                h0, h1 = 2 * p, 2 * p + 1
                nc.tensor.matmul(
                    av0, vaug[kt][:, h0 * VP:h0 * VP + VP], et[:, 0:512],
                    start=(kt == 0), stop=(kt == KT - 1),
                )
                nc.tensor.matmul(
                    av1, vaug[kt][:, h1 * VP:h1 * VP + VP], et[:, 512:1024],
                    start=(kt == 0), stop=(kt == KT - 1),
                )

            def attn(p, qc, qts, kts, extra=None):
                """Software-pipelined kt loop: scores/exp lead av by SKEW."""
                h0, h1 = 2 * p, 2 * p + 1
                av0 = av_ps.tile([VP, 512], F32, tag="av", name=f"av{h0}_{qc}")
                av1 = av_ps.tile([VP, 512], F32, tag="av", name=f"av{h1}_{qc}")
                ets = [None] * KT
                for kt in range(KT):
                    if extra is not None:
                        extra(kt)
                    ets[kt] = score_exp(p, qc, qts, kts, kt)
                    if kt >= SKEW:
                        av_step(p, av0, av1, ets[kt - SKEW], kt - SKEW)
                        ets[kt - SKEW] = None
                for kt in range(KT - SKEW, KT):
                    av_step(p, av0, av1, ets[kt], kt)
                # drain + normalize, off the PE critical path
                for hi, av in ((0, av0), (1, av1)):
                    h = 2 * p + hi
                    avs = avs_pool.tile([VP, 512], F32, tag="avs",
                                        name=f"avs{h}_{qc}")
                    nc.vector.tensor_copy(avs, av)
                    # row 64 = softmax denominators; shift to partition 0
                    sm0 = iv_pool.tile([1, 512], F32, tag="sm0",
                                       name=f"sm0{h}_{qc}")
                    nc.gpsimd.dma_start(out=sm0, in_=avs[HD:VP, :])
                    iv0 = iv_pool.tile([1, 512], F32, tag="iv0",
                                       name=f"iv0{h}_{qc}")
                    nc.vector.reciprocal_approx_fast(out=iv0, in_=sm0)
                    bc = bc_pool.tile([64, 512], F32, tag="bc",
                                      name=f"bc{h}_{qc}")
                    nc.gpsimd.partition_broadcast(bc, iv0)
                    if hi == 0:
                        nc.vector.tensor_mul(
                            outT[p][qc][0:64, :], avs[0:HD, :], bc)
                    else:
                        ost = ost_pool.tile([64, 512], BF16, tag="ost",
                                            name=f"ost{h}_{qc}")
                        nc.vector.tensor_mul(ost, avs[0:HD, :], bc)
                        nc.gpsimd.dma_start(out=outT[p][qc][64:128, :],
                                            in_=ost)

            def proj(tt, wptt):
                ysb = y_pool.tile([128, C], BF16, tag="y", name=f"y{tt}")
                for ec in range(EC):
                    py = proj_ps.tile([128, 384], F32, tag="mmps",
                                      name=f"py{tt}_{ec}")
                    for c in range(CT):
                        nc.tensor.matmul(
                            py,
                            outT[c][tt // 4][:, (tt % 4) * 128:(tt % 4 + 1) * 128],
                            wptt[c][:, ec * 384:(ec + 1) * 384],
                            start=(c == 0), stop=(c == CT - 1),
                        )
                    nc.vector.tensor_add(ysb[:, ec * 384:(ec + 1) * 384], py,
                                         bias_bc[:, ec * 384:(ec + 1) * 384])
                nc.sync.dma_start(out=y[tt * 128:(tt + 1) * 128, :], in_=ysb)

            # ---- pipeline ------------------------------------------------
            # pair 0 q/k projection, then qc0 attention interleaved with the
            # v-projection (vaug[kt] is produced just before its av matmul).
            cur = qk_project(0)
            qts, kts = cur
            attn(0, 0, qts, kts, extra=vproj)

            nxt = qk_project(1)
            attn(0, 1, qts, kts)

            wptt = None
            for p in range(1, P):
                qts, kts = nxt
                qc_order = (1, 0) if p == P - 1 else (0, 1)
                attn(p, qc_order[0], qts, kts)

                # next pair's projection between the two query halves
                if p < P - 1:
                    nxt = qk_project(p + 1)
                if p == 2:
                    # prefetch output-projection weights mid-flight
                    wptt = []
                    for c in range(CT):
                        t = w768_pool.tile([128, C], BF16, tag="w768",
                                           name=f"wpt{c}")
                        nc.sync.dma_start(out=t, in_=wpt[c, :, :])
                        wptt.append(t)

                attn(p, qc_order[1], qts, kts)

            # ---- output projection: pair 5 finished qc1 first, so tokens
            # 512-1023 project during its qc0 attention; tokens 0-511 last -
            for tt in list(range(4, KT)) + list(range(4)):
                proj(tt, wptt)

    nc.finalize()
    return nc


def _get_nc():
    global _NC_CACHE
    if _NC_CACHE is None:
        _NC_CACHE = _build()
    return _NC_CACHE


def _host_prep(x, Wqkv, Wproj, bproj, Aq, Bq, Av, Bv):
    """Fold LoRA + score scale into the weights; lay out and cast to bf16."""
    bf16 = ml_dtypes.bfloat16
    W = Wqkv.astype(np.float64)
    Wq = W[0:C].reshape(H, HD, C)
    Wk = W[C:2 * C].reshape(H, HD, C)
    Wv_ = W[2 * C:3 * C].reshape(H, HD, C)
    ABq = Aq.astype(np.float64) @ Bq.astype(np.float64)   # [HD, HD]
    ABv = Av.astype(np.float64) @ Bv.astype(np.float64)
    Wq = Wq + np.einsum('ed,hec->hdc', ABq, Wq)           # (I+AB).T @ Wq per head
    Wv_ = Wv_ + np.einsum('ed,hec->hdc', ABv, Wv_)
    Wq = Wq * (HD ** -0.5)                                # fold score scale

    # wq/wk[c] = [K=c-rows(128), 768 = 12 heads x 64 dims, pair-major]
    wq_ = np.empty((CT, 128, C), np.float32)
    wk_ = np.empty((CT, 128, C), np.float32)
    for h in range(H):
        for c in range(CT):
            cs = slice(c * 128, (c + 1) * 128)
            wq_[c, :, h * 64:(h + 1) * 64] = Wq[h][:, cs].T.astype(np.float32)
            wk_[c, :, h * 64:(h + 1) * 64] = Wk[h][:, cs].T.astype(np.float32)

    # wv[c] = [K=c-rows(128), all 768 v output features]
    WvT = Wv_.reshape(C, C).T.astype(np.float32)          # [c_in, v_out]
    wv_ = np.ascontiguousarray(WvT.reshape(CT, 128, C))

    # wpt[c] = Wproj.T c-tiles: [K=c(128), e(768)]
    WpT = Wproj.astype(np.float32).T                      # [c, e]
    wpt_ = np.ascontiguousarray(WpT.reshape(CT, 128, C))

    bias_ = bproj.astype(np.float32).reshape(1, C)

    wq16 = wq_.astype(bf16)
    wk16 = wk_.astype(bf16)
    wv16 = wv_.astype(bf16)
    wpt16 = wpt_.astype(bf16)

    per_core = []
    for b in range(B):
        xTb = np.ascontiguousarray(x[b].astype(np.float32).T).astype(bf16)
        per_core.append({"xT": xTb, "wq": wq16, "wk": wk16, "wv": wv16,
                         "wpt": wpt16, "bias": bias_})
    return per_core


def kernel(x, Wqkv, Wproj, bproj, Aq, Bq, Av, Bv, _trace=False):
    x = np.asarray(x)
    in_maps = _host_prep(np.asarray(x), np.asarray(Wqkv), np.asarray(Wproj),
                         np.asarray(bproj), np.asarray(Aq), np.asarray(Bq),
                         np.asarray(Av), np.asarray(Bv))
    nc = _get_nc()
    res = run_bass_kernel_spmd(nc, in_maps, core_ids=list(range(N_CORES)),
                               trace=_trace)
    out = np.stack([res.results[b]["y"] for b in range(B)], axis=0)
    if _trace:
        kernel._last_result = res
    return out.astype(np.float32)


# revision 5
# speedup vs baseline: 1.0095x; 1.0095x over previous
"""LoRA attention kernel for Trainium2, batch-sharded across 8 NeuronCores.

Strategy (v5):
  - Data parallel: batch B=8 -> one batch element per core.
  - LoRA factors folded into Wqkv on the host (exact algebra, float64);
    the 1/sqrt(hd) score scale is folded into Wq as well.
  - All matmul operands are bfloat16 (1 cycle/row on the PE regardless of
    moving free dim, and FWL halves LDWEIGHTS time); accumulation is fp32
    in PSUM so only operand rounding is lost.
  - Heads are processed in pairs (2p, 2p+1). qT/kT tiles pack the pair's
    head dims on partitions [0:64] / [64:128]. The score matmuls are ROW
    TILED: two concurrent 64-row matmuls (tile_position auto-derived from
    base partitions) compute both heads' scores in one 512-column pass.
  - The pair's two score outputs go to one [128,1024] 2-bank PSUM tile so
    a single ACT instruction applies exp to both (halves ACT bubbles).
    The exp stream is the pacing engine (~107us); everything else is
    arranged to hide under it:
      * each input tensor lives in one wide SBUF tile filled by ONE DMA
        (descriptor posting costs ~0.6us per dma_start — the v4 lead-in
        was posting-bound); a single InstDMACopy is split across all 16
        SDMA engines, so big posts still get full bandwidth. Posts are
        balanced across the sync (HWDGE) and gpsimd (SWDGE) rings.
      * v-projection groups are interleaved with pair-0/qc-0 attention —
        the attention-value matmul for key tile kt only needs vaug[kt].
      * qk-projection for pair p+1 is emitted between the two query
        halves of pair p.
      * the attention kt loop is software-pipelined: scores/exp run
        SKEW=3 key tiles ahead of the attention-value matmuls, so the
        av PSUM bank turnaround at (pair, qc) boundaries does not stall
        the in-order PE queue.
      * pair 5 handles its second query half first; the output
        projection for those tokens is interleaved into pair-5/qc-0's
        kt loop, so only the last quarter of the projection is a tail.
  - v is produced in natural layout with an extra all-ones column per head
    (65-col pitch); the attention-value matmul (M=65) accumulates softmax
    denominators for free in its last output row.
  - Normalization runs off the PE critical path: DVE drains av PSUM, a
    DMA shifts the denominator row to partition 0, fast reciprocal,
    gpsimd partition-broadcast, DVE multiply into outT.
  - y is written back as bf16 (cast to f32 on host).
"""
import numpy as np
import ml_dtypes

import concourse.bass as bass
import concourse.bacc as bacc
import concourse.mybir as mybir
import concourse.tile as tile
from concourse.bass_utils import run_bass_kernel_spmd

F32 = mybir.dt.float32
BF16 = mybir.dt.bfloat16
EXP = mybir.ActivationFunctionType.Exp

B, N, C, H, HD = 8, 1024, 768, 12, 64
P = H // 2              # 6 head pairs
CT = C // 128           # 6 contraction tiles over C
QC = N // 512           # 2 query chunks of 512
KT = N // 128           # 8 key tiles of 128
EC = 2                  # output-projection feature chunks of 384
VP = HD + 1             # 65-col pitch per head in vaug
SKEW = 3                # score/exp lead over av in the kt pipeline
N_CORES = 8

_NC_CACHE = None


def _build():
    nc = bacc.Bacc(None, target_bir_lowering=False)

    xT = nc.dram_tensor("xT", [C, N], BF16, kind="ExternalInput")
    # wq/wk[c] = [K=c-rows(128), 6 pairs x (head 2p dims 64 | head 2p+1 dims 64)]
    wq = nc.dram_tensor("wq", [CT, 128, C], BF16, kind="ExternalInput")
    wk = nc.dram_tensor("wk", [CT, 128, C], BF16, kind="ExternalInput")
    wv = nc.dram_tensor("wv", [CT, 128, C], BF16, kind="ExternalInput")
    wpt = nc.dram_tensor("wpt", [CT, 128, C], BF16, kind="ExternalInput")
    bias = nc.dram_tensor("bias", [1, C], F32, kind="ExternalInput")
    y = nc.dram_tensor("y", [N, C], BF16, kind="ExternalOutput")

    from contextlib import ExitStack
    with tile.TileContext(nc) as tc:
        with ExitStack() as ctx:
            pool = lambda name, bufs, **kw: ctx.enter_context(
                tc.tile_pool(name=name, bufs=bufs, **kw))
            vaug_pool = pool("vaug", KT)
            qt_pool = pool("qtp", 4)              # qT pair tiles, 2 pairs
            kt_pool = pool("ktp", 4)
            et_pool = pool("expp", 6)
            avs_pool = pool("avsp", 4)
            iv_pool = pool("ivp", 4)
            bc_pool = pool("bcp", 3)
            ost_pool = pool("ostp", 3)
            out_pool = pool("outp", 2 * CT)
            y_pool = pool("yp", 3)
            cst_pool = pool("cst", 1)
            proj_ps = pool("proj_ps", 2, space="PSUM")
            sc_ps = pool("sc_ps", 2, space="PSUM")
            av_ps = pool("av_ps", 2, space="PSUM")

            # ---- PE warm-up: dummy matmuls bridge the DMA lead-in so the
            # HAM clock gate opens before real work arrives -----------------
            wuf = cst_pool.tile([128, 512], F32, tag="wuf")
            nc.vector.memset(wuf, 0.0)
            wur = cst_pool.tile([128, 512], BF16, tag="wur")
            nc.vector.tensor_copy(wur, wuf)

            for i in range(8):
                wps = proj_ps.tile([128, 512], F32, tag="mmps",
                                   name=f"wu_{i}")
                nc.tensor.matmul(wps, wur[:, 0:128], wur,
                                 start=True, stop=True)

            # ---- loads: one wide SBUF tile + one DMA per tensor ----------
            # dram [CT, 128, X] -> sbuf [128, CT * X], c-major along free dim
            def cmajor_ap(dram, X, n0=0, n1=None):
                n1 = X if n1 is None else n1
                return bass.AP(
                    tensor=dram.tensor if isinstance(dram, bass.AP) else dram,
                    offset=n0,
                    ap=[[X, 128], [128 * X, CT], [1, n1 - n0]])

            wq_all = cst_pool.tile([128, CT * C], BF16, tag="wq_all")
            nc.sync.dma_start(out=wq_all, in_=cmajor_ap(wq[:, :, :], C))
            xt_all = cst_pool.tile([128, CT * N], BF16, tag="xt_all")
            # qc0 token halves of every c-chunk first (needed by first scores)
            for qc in range(QC):
                dst = bass.AP(tensor=xt_all.tensor,
                              offset=xt_all.offset + qc * 512,
                              ap=[xt_all.ap[0], [N, CT], [1, 512]])
                nc.sync.dma_start(
                    out=dst, in_=cmajor_ap(xT[:, :], N, qc * 512,
                                           (qc + 1) * 512))
            wk_all = cst_pool.tile([128, CT * C], BF16, tag="wk_all")
            nc.gpsimd.dma_start(out=wk_all, in_=cmajor_ap(wk[:, :, :], C))
            wv_all = cst_pool.tile([128, CT * C], BF16, tag="wv_all")
            nc.gpsimd.dma_start(out=wv_all, in_=cmajor_ap(wv[:, :, :], C))
            bias_bc = cst_pool.tile([128, C], F32, tag="biasbc")
            nc.gpsimd.dma_start(out=bias_bc,
                                in_=bias[:, :].to_broadcast([128, C]))

            def xt(c, qc):      # [128, 512] moving slice, contiguous rows
                return xt_all[:, c * N + qc * 512:c * N + (qc + 1) * 512]

            def xt128(c, tt):   # [128, 128] stationary slice
                return xt_all[:, c * N + tt * 128:c * N + (tt + 1) * 128]

            ones12 = cst_pool.tile([128, H], BF16, tag="ones12")
            nc.vector.memset(ones12, 1.0)

            # ---- per-pair q/k projection (qc0 groups first so the first
            # scores can start two groups earlier) -------------------------
            def qk_project(p):
                """qT_pair / kT_pair: [128 = (h dims | h' dims), 512 tokens]
                per 512-token chunk, bf16."""
                qts, kts = [None] * QC, [None] * QC
                for qc in range(QC):
                    for qk in range(2):
                        dst_pool = qt_pool if qk == 0 else kt_pool
                        w_all = wq_all if qk == 0 else wk_all
                        st = dst_pool.tile([128, 512], BF16, tag="st",
                                           name=f"st{p}_{qk}_{qc}")
                        pqk = proj_ps.tile([128, 512], F32, tag="mmps",
                                           name=f"pqk{p}_{qk}_{qc}")
                        for c in range(CT):
                            nc.tensor.matmul(
                                pqk,
                                w_all[:, c * C + p * 128:c * C + (p + 1) * 128],
                                xt(c, qc),
                                start=(c == 0), stop=(c == CT - 1),
                            )
                        nc.vector.tensor_copy(st, pqk)
                        (qts if qk == 0 else kts)[qc] = st
                return qts, kts

            # ---- v_aug[kt] group emission (natural layout, [v | 1]) ------
            vaug = [None] * KT

            def vproj(tt):
                va = vaug_pool.tile([128, H * VP], BF16,
                                    tag="vaug", name=f"vaug{tt}")
                for half in range(2):
                    pv = proj_ps.tile([128, 384], F32, tag="mmps",
                                      name=f"pv{tt}_{half}")
                    for c in range(CT):
                        nc.tensor.matmul(
                            pv, xt128(c, tt),
                            wv_all[:, c * C + half * 384:c * C + (half + 1) * 384],
                            start=(c == 0), stop=(c == CT - 1),
                        )
                    dst = bass.AP(tensor=va.tensor,
                                  offset=va.offset + half * 6 * VP,
                                  ap=[va.ap[0], [VP, 6], [1, HD]])
                    nc.vector.tensor_copy(dst, pv)
                ones_ap = bass.AP(tensor=va.tensor, offset=va.offset + HD,
                                  ap=[va.ap[0], [VP, H]])
                nc.vector.tensor_copy(ones_ap, ones12)
                vaug[tt] = va

            # ---- output accumulator tiles (c-major, [128, 512] per qc) ---
            outT = [[out_pool.tile([128, 512], BF16, tag="outT",
                                   name=f"outT{i}_{qc}")
                     for qc in range(QC)] for i in range(CT)]

            # ---- attention building blocks -------------------------------
            def score_exp(p, qc, qts, kts, kt):
                ps_s = sc_ps.tile([128, 1024], F32, tag="sc",
                                  name=f"sc{p}_{qc}_{kt}")
                klhs = kts[kt // 4][:, (kt % 4) * 128:(kt % 4 + 1) * 128]
                nc.tensor.matmul(
                    ps_s[:, 0:512], klhs[0:64, :], qts[qc][0:64, :],
                    start=True, stop=True,
                )
                nc.tensor.matmul(
                    ps_s[:, 512:1024], klhs[64:128, :], qts[qc][64:128, :],
                    start=True, stop=True,
                )
                et = et_pool.tile([128, 1024], BF16, tag="exp",
                                  name=f"exp{p}_{qc}_{kt}")
                nc.scalar.activation(out=et, in_=ps_s, func=EXP)
                return et

            def av_step(p, av0, av1, et, kt):
                h0, h1 = 2 * p, 2 * p + 1
                nc.tensor.matmul(
                    av0, vaug[kt][:, h0 * VP:h0 * VP + VP], et[:, 0:512],
                    start=(kt == 0), stop=(kt == KT - 1),
                )
                nc.tensor.matmul(
                    av1, vaug[kt][:, h1 * VP:h1 * VP + VP], et[:, 512:1024],
                    start=(kt == 0), stop=(kt == KT - 1),
                )

            def attn(p, qc, qts, kts, extra=None):
                """Software-pipelined kt loop: scores/exp lead av by SKEW."""
                h0, h1 = 2 * p, 2 * p + 1
                av0 = av_ps.tile([VP, 512], F32, tag="av", name=f"av{h0}_{qc}")
                av1 = av_ps.tile([VP, 512], F32, tag="av", name=f"av{h1}_{qc}")
                ets = [None] * KT
                for kt in range(KT):
                    if extra is not None:
                        extra(kt)
                    ets[kt] = score_exp(p, qc, qts, kts, kt)
                    if kt >= SKEW:
                        av_step(p, av0, av1, ets[kt - SKEW], kt - SKEW)
                        ets[kt - SKEW] = None
                for kt in range(KT - SKEW, KT):
                    av_step(p, av0, av1, ets[kt], kt)
                # drain + normalize, off the PE critical path
                for hi, av in ((0, av0), (1, av1)):
                    h = 2 * p + hi
                    avs = avs_pool.tile([VP, 512], F32, tag="avs",
                                        name=f"avs{h}_{qc}")
                    nc.vector.tensor_copy(avs, av)
                    # row 64 = softmax denominators; shift to partition 0
                    sm0 = iv_pool.tile([1, 512], F32, tag="sm0",
                                       name=f"sm0{h}_{qc}")
                    nc.sync.dma_start(out=sm0, in_=avs[HD:VP, :])
                    iv0 = iv_pool.tile([1, 512], F32, tag="iv0",
                                       name=f"iv0{h}_{qc}")
                    nc.vector.reciprocal_approx_fast(out=iv0, in_=sm0)
                    bc = bc_pool.tile([64, 512], F32, tag="bc",
                                      name=f"bc{h}_{qc}")
                    nc.gpsimd.partition_broadcast(bc, iv0)
                    if hi == 0:
                        nc.vector.tensor_mul(
                            outT[p][qc][0:64, :], avs[0:HD, :], bc)
                    else:
                        ost = ost_pool.tile([64, 512], BF16, tag="ost",
                                            name=f"ost{h}_{qc}")
                        nc.vector.tensor_mul(ost, avs[0:HD, :], bc)
                        nc.sync.dma_start(out=outT[p][qc][64:128, :],
                                          in_=ost)

            wptt = cst_pool.tile([128, CT * C], BF16, tag="wptt")

            def proj_group(tt, ec, ysb):
                py = proj_ps.tile([128, 384], F32, tag="mmps",
                                  name=f"py{tt}_{ec}")
                for c in range(CT):
                    nc.tensor.matmul(
                        py,
                        outT[c][tt // 4][:, (tt % 4) * 128:(tt % 4 + 1) * 128],
                        wptt[:, c * C + ec * 384:c * C + (ec + 1) * 384],
                        start=(c == 0), stop=(c == CT - 1),
                    )
                nc.vector.tensor_add(ysb[:, ec * 384:(ec + 1) * 384], py,
                                     bias_bc[:, ec * 384:(ec + 1) * 384])

            def proj(tt):
                ysb = y_pool.tile([128, C], BF16, tag="y", name=f"y{tt}")
                for ec in range(EC):
                    proj_group(tt, ec, ysb)
                nc.sync.dma_start(out=y[tt * 128:(tt + 1) * 128, :], in_=ysb)

            # ---- pipeline ------------------------------------------------
            # pair 0 q/k projection, then qc0 attention interleaved with the
            # v-projection (vaug[kt] is produced just before its av matmul).
            qts, kts = qk_project(0)
            attn(0, 0, qts, kts, extra=vproj)

            nxt = qk_project(1)
            attn(0, 1, qts, kts)

            for p in range(1, P - 1):
                qts, kts = nxt
                attn(p, 0, qts, kts)
                nxt = qk_project(p + 1)
                if p == 2:
                    # prefetch output-projection weights mid-flight
                    nc.sync.dma_start(out=wptt,
                                      in_=cmajor_ap(wpt[:, :, :], C))
                attn(p, 1, qts, kts)

            # pair 5: qc1 first; its tokens' projection (tt 4-7) interleaves
            # with the qc0 kt loop, leaving only tt 0-3 as the tail.
            qts, kts = nxt
            attn(P - 1, 1, qts, kts)

            ysb_h = {}

            def proj_extra(kt):
                tt = 4 + kt // 2
                ec = kt % 2
                if ec == 0:
                    ysb_h[tt] = y_pool.tile([128, C], BF16, tag="y",
                                            name=f"y{tt}")
                proj_group(tt, ec, ysb_h[tt])
                if ec == 1:
                    nc.sync.dma_start(out=y[tt * 128:(tt + 1) * 128, :],
                                      in_=ysb_h[tt])

            attn(P - 1, 0, qts, kts, extra=proj_extra)

            for tt in range(4):
                proj(tt)

    nc.finalize()
    return nc


def _get_nc():
    global _NC_CACHE
    if _NC_CACHE is None:
        _NC_CACHE = _build()
    return _NC_CACHE


def _host_prep(x, Wqkv, Wproj, bproj, Aq, Bq, Av, Bv):
    """Fold LoRA + score scale into the weights; lay out and cast to bf16."""
    bf16 = ml_dtypes.bfloat16
    W = Wqkv.astype(np.float64)
    Wq = W[0:C].reshape(H, HD, C)
    Wk = W[C:2 * C].reshape(H, HD, C)
    Wv_ = W[2 * C:3 * C].reshape(H, HD, C)
    ABq = Aq.astype(np.float64) @ Bq.astype(np.float64)   # [HD, HD]
    ABv = Av.astype(np.float64) @ Bv.astype(np.float64)
    Wq = Wq + np.einsum('ed,hec->hdc', ABq, Wq)           # (I+AB).T @ Wq per head
    Wv_ = Wv_ + np.einsum('ed,hec->hdc', ABv, Wv_)
    Wq = Wq * (HD ** -0.5)                                # fold score scale

    # wq/wk[c] = [K=c-rows(128), 768 = 12 heads x 64 dims, head-major]
    wq_ = np.empty((CT, 128, C), np.float32)
    wk_ = np.empty((CT, 128, C), np.float32)
    for h in range(H):
        for c in range(CT):
            cs = slice(c * 128, (c + 1) * 128)
            wq_[c, :, h * 64:(h + 1) * 64] = Wq[h][:, cs].T.astype(np.float32)
            wk_[c, :, h * 64:(h + 1) * 64] = Wk[h][:, cs].T.astype(np.float32)

    # wv[c] = [K=c-rows(128), all 768 v output features]
    WvT = Wv_.reshape(C, C).T.astype(np.float32)          # [c_in, v_out]
    wv_ = np.ascontiguousarray(WvT.reshape(CT, 128, C))

    # wpt[c] = Wproj.T c-tiles: [K=c(128), e(768)]
    WpT = Wproj.astype(np.float32).T                      # [c, e]
    wpt_ = np.ascontiguousarray(WpT.reshape(CT, 128, C))

    bias_ = bproj.astype(np.float32).reshape(1, C)

    wq16 = wq_.astype(bf16)
    wk16 = wk_.astype(bf16)
    wv16 = wv_.astype(bf16)
    wpt16 = wpt_.astype(bf16)

    per_core = []
    for b in range(B):
        xTb = np.ascontiguousarray(x[b].astype(np.float32).T).astype(bf16)
        per_core.append({"xT": xTb, "wq": wq16, "wk": wk16, "wv": wv16,
                         "wpt": wpt16, "bias": bias_})
    return per_core


def kernel(x, Wqkv, Wproj, bproj, Aq, Bq, Av, Bv, _trace=False):
    x = np.asarray(x)
    in_maps = _host_prep(np.asarray(x), np.asarray(Wqkv), np.asarray(Wproj),
                         np.asarray(bproj), np.asarray(Aq), np.asarray(Bq),
                         np.asarray(Av), np.asarray(Bv))
    nc = _get_nc()
    res = run_bass_kernel_spmd(nc, in_maps, core_ids=list(range(N_CORES)),
                               trace=_trace)
    out = np.stack([res.results[b]["y"] for b in range(B)], axis=0)
    if _trace:
        kernel._last_result = res
    return out.astype(np.float32)
